# revision 14
# baseline (speedup 1.0000x reference)
"""Trainium2 Bass kernel for nn_DecoderFCWithCuboic.

Data-parallel over 8 NeuronCores: batch 4096 -> 512 rows/core, MLP weights
replicated. Per core:
  points branch: h1 = lrelu(x@W1+b1); h2 = lrelu(h1@W2+b2);
                 pts = sigmoid(h2@W3+b3)            (512, 6144)
  cuboid branch: cub = sigmoid(lrelu(lrelu(x@cW1+cb1)@cW2+cb2)@cW3+cb3)
  cuboid distance:
     per coord: m = min(s, c-s)  (= c/2 - |s - c/2|), o = s*[s>c]
     minn = mean_p( relu(min_xyz m) + max_xyz o )
  identities used on-device:
     min(s, c-s) = c/2 - |s - c/2|
     relu(min_xyz m) = relu(-max_xyz(|s-c/2| - c/2))
     max_xyz(s*[s>c]) is 0 when all coords are inliers, so no inlier-indicator
     select is needed: result = relu(min3) + max3(o).
"""

import os
import sys

import numpy as np

for _p in ("/opt/trn_rl_repo", "/root/.axon_site/_ro/trn_rl_repo"):
    if os.path.isdir(_p) and _p not in sys.path:
        sys.path.insert(0, _p)

import concourse.bass as bass
import concourse.mybir as mybir
import concourse.tile as tile
from concourse import bacc
from concourse.bass import ds, ts
from concourse.bass_utils import run_bass_kernel_spmd

BZ, D, H, NP = 4096, 128, 256, 2048  # batch, in-dim, hidden, points
NCORES = 8
BZC = BZ // NCORES  # 512 batch rows per core
NF = 3 * NP  # 6144 point features
F32 = mybir.dt.float32
EW = mybir.dt.bfloat16  # dtype for cuboid-distance intermediates
AF = mybir.ActivationFunctionType
OP = mybir.AluOpType
SLOPE = 0.01  # torch LeakyReLU default


def _lrelu(nc, pool, z_psum, out_sbuf, bias_col):
    """out = lrelu(z + bias). ACT moves PSUM->SBUF with the per-partition bias
    folded in; DVE then applies max(x, SLOPE*x) in one fused op."""
    zsb = pool.tile(list(z_psum.shape), F32, tag="zsb")
    nc.scalar.activation(zsb, z_psum, AF.Identity, bias=bias_col, scale=1.0)
    nc.vector.scalar_tensor_tensor(
        out=out_sbuf, in0=zsb, scalar=SLOPE, in1=zsb, op0=OP.mult, op1=OP.max
    )


def _build(with_b3: bool, with_cb3: bool, n_rows: int = BZC):
    """Build the single-core SPMD program. n_rows must be a multiple of 128."""
    nc = bacc.Bacc()
    nbt = n_rows // 128  # batch sub-tiles

    xT_d = nc.dram_tensor("xT", [D, n_rows], F32, kind="ExternalInput")
    W1_d = nc.dram_tensor("W1", [D, H], F32, kind="ExternalInput")
    b1_d = nc.dram_tensor("b1", [H], F32, kind="ExternalInput")
    W2_d = nc.dram_tensor("W2", [H, H], F32, kind="ExternalInput")
    b2_d = nc.dram_tensor("b2", [H], F32, kind="ExternalInput")
    W3_d = nc.dram_tensor("W3", [H, NF], F32, kind="ExternalInput")
    b3_d = nc.dram_tensor("b3", [1, NF], F32, kind="ExternalInput")
    cW1_d = nc.dram_tensor("cW1", [D, H], F32, kind="ExternalInput")
    cb1_d = nc.dram_tensor("cb1", [H], F32, kind="ExternalInput")
    cW2_d = nc.dram_tensor("cW2", [H, H], F32, kind="ExternalInput")
    cb2_d = nc.dram_tensor("cb2", [H], F32, kind="ExternalInput")
    cW3_d = nc.dram_tensor("cW3", [H, 3], F32, kind="ExternalInput")
    cb3_d = nc.dram_tensor("cb3", [1, 3], F32, kind="ExternalInput")

    pts_d = nc.dram_tensor("pts", [n_rows, 3, NP], F32, kind="ExternalOutput")
    minn_d = nc.dram_tensor("minn", [n_rows, 1], F32, kind="ExternalOutput")
    cub_d = nc.dram_tensor("cub", [n_rows, 3], F32, kind="ExternalOutput")

    from contextlib import ExitStack

    with tile.TileContext(nc) as tc, ExitStack() as ctx:
        consts = ctx.enter_context(tc.tile_pool(name="consts", bufs=1))

        # ---- resident weights / biases -------------------------------------
        xT = consts.tile([D, n_rows], F32)
        nc.sync.dma_start(xT, xT_d[:, :])
        W1 = consts.tile([128, H], F32)
        nc.sync.dma_start(W1, W1_d[:, :])
        W2 = consts.tile([128, 2, H], F32)
        nc.sync.dma_start(W2, W2_d[:, :].rearrange("(k p) m -> p k m", p=128))
        W3 = consts.tile([128, 2, NF], F32)
        nc.sync.dma_start(W3, W3_d[:, :].rearrange("(k p) n -> p k n", p=128))
        cW1 = consts.tile([128, H], F32)
        nc.sync.dma_start(cW1, cW1_d[:, :])
        cW2 = consts.tile([128, 2, H], F32)
        nc.sync.dma_start(cW2, cW2_d[:, :].rearrange("(k p) m -> p k m", p=128))
        cW3 = consts.tile([128, 2, 3], F32)
        nc.sync.dma_start(cW3, cW3_d[:, :].rearrange("(k p) n -> p k n", p=128))
        b1 = consts.tile([128, 2], F32)
        nc.sync.dma_start(b1, b1_d[:].rearrange("(m p) -> p m", p=128))
        b2 = consts.tile([128, 2], F32)
        nc.sync.dma_start(b2, b2_d[:].rearrange("(m p) -> p m", p=128))
        cb1 = consts.tile([128, 2], F32)
        nc.sync.dma_start(cb1, cb1_d[:].rearrange("(m p) -> p m", p=128))
        cb2 = consts.tile([128, 2], F32)
        nc.sync.dma_start(cb2, cb2_d[:].rearrange("(m p) -> p m", p=128))
        cb3 = consts.tile([1, 3], F32)
        nc.sync.dma_start(cb3, cb3_d[:, :])
        ones = consts.tile([1, 128], F32)
        nc.vector.memset(ones, 1.0)
        z0 = consts.tile([128, NP], EW)
        nc.vector.memset(z0, 0.0)
        b3 = None
        if with_b3:
            b3 = consts.tile([1, NF], F32)
            nc.sync.dma_start(b3, b3_d[:, :])

        # ---- layer phase: h1/h2/c1/c2, feature-major [feat, batch] ---------
        hpool = ctx.enter_context(tc.tile_pool(name="hpool", bufs=1))
        zpool = ctx.enter_context(tc.tile_pool(name="zpool", bufs=2))
        h2 = hpool.tile([128, 2, n_rows], F32)
        c2 = hpool.tile([128, 2, n_rows], F32)
        h1 = hpool.tile([128, 2, n_rows], F32)
        c1 = hpool.tile([128, 2, n_rows], F32)
        with tc.tile_pool(name="lay_psum", bufs=4, space="PSUM") as lpsum:
            for W_in, b_in, out_t in ((W1, b1, h1), (cW1, cb1, c1)):
                for m in range(2):
                    zp = lpsum.tile([128, n_rows], F32, tag="zp")
                    nc.tensor.matmul(
                        zp, W_in[:, ts(m, 128)], xT[:, :], start=True, stop=True
                    )
                    _lrelu(nc, zpool, zp, out_t[:, m, :], b_in[:, m : m + 1])
            for W_in, b_in, in_t, out_t in ((W2, b2, h1, h2), (cW2, cb2, c1, c2)):
                for m in range(2):
                    zp = lpsum.tile([128, n_rows], F32, tag="zp")
                    for k in range(2):
                        nc.tensor.matmul(
                            zp,
                            W_in[:, k, ts(m, 128)],
                            in_t[:, k, :],
                            start=(k == 0),
                            stop=(k == 1),
                        )
                    _lrelu(nc, zpool, zp, out_t[:, m, :], b_in[:, m : m + 1])

        # ---- per-sub-tile phase --------------------------------------------
        spool = ctx.enter_context(tc.tile_pool(name="spool", bufs=2))
        sbpool = ctx.enter_context(tc.tile_pool(name="sbpool", bufs=2))
        dpool = ctx.enter_context(tc.tile_pool(name="dpool", bufs=3))
        opool = ctx.enter_context(tc.tile_pool(name="opool", bufs=3))
        tpool = ctx.enter_context(tc.tile_pool(name="tpool", bufs=2))
        cpool = ctx.enter_context(tc.tile_pool(name="cpool", bufs=2))
        smpool = ctx.enter_context(tc.tile_pool(name="smpool", bufs=4))
        pts_psum = ctx.enter_context(
            tc.tile_pool(name="pts_psum", bufs=3, space="PSUM")
        )
        cub_psum = ctx.enter_context(
            tc.tile_pool(name="cub_psum", bufs=2, space="PSUM")
        )

        for bt in range(nbt):
            bsl = ts(bt, 128)
            # cuboid head: cub[128, 3] (batch-partition layout)
            cp = cub_psum.tile([128, 3], F32, tag="cp")
            if with_cb3:
                nc.tensor.matmul(cp, ones, cb3, start=True, stop=False)
            for k in range(2):
                nc.tensor.matmul(
                    cp,
                    c2[:, k, bsl],
                    cW3[:, k, :],
                    start=(k == 0 and not with_cb3),
                    stop=(k == 1),
                )
            cub_s = cpool.tile([128, 3], F32, tag="cub")
            nc.scalar.activation(cub_s, cp, AF.Sigmoid)
            nc.sync.dma_start(cub_d[bsl, :], cub_s)

            # pts matmuls + sigmoid: s [128, 3, 2048] f32, batch-partition
            s = spool.tile([128, 3, NP], F32, tag="s")
            for w in range(3):
                for half in range(2):
                    pp = pts_psum.tile([128, 1024], F32, tag="pp")
                    for nn in range(2):
                        col0 = w * NP + half * 1024 + nn * 512
                        if with_b3:
                            nc.tensor.matmul(
                                pp[:, ts(nn, 512)],
                                ones,
                                b3[:, ds(col0, 512)],
                                start=True,
                                stop=False,
                            )
                        for k in range(2):
                            nc.tensor.matmul(
                                pp[:, ts(nn, 512)],
                                h2[:, k, bsl],
                                W3[:, k, ds(col0, 512)],
                                start=(k == 0 and not with_b3),
                                stop=(k == 1),
                            )
                    nc.scalar.activation(
                        s[:, w, ds(half * 1024, 1024)], pp, AF.Sigmoid
                    )
            nc.sync.dma_start(pts_d[bsl, :, :], s)

            # cuboid distance.
            #   d_w = s_w - c_w  (sign(d) = inlier/outlier mask, exact in bf16)
            #   min3 m = -B with B = max(max3(d), -min3(s))
            #   minn = (sum relu(-B) + sum max3(s*[d>0])) / NP
            # GPSIMD: bf16 shadow of s + the three d tensors. DVE: the rest,
            # all bf16 1-port ops (no shared-port contention with GPSIMD).
            s_b = sbpool.tile([128, 3, NP], EW, tag="sb")
            nc.gpsimd.tensor_copy(out=s_b, in_=s)
            d = []
            for w in range(3):
                d_t = dpool.tile([128, NP], EW, tag="d")
                nc.gpsimd.tensor_scalar(
                    out=d_t,
                    in0=s[:, w, :],
                    scalar1=cub_s[:, w : w + 1],
                    scalar2=None,
                    op0=OP.subtract,
                )
                d.append(d_t)

            u2 = tpool.tile([128, NP], EW, tag="u")
            nc.vector.tensor_tensor(u2, d[0], d[1], OP.max)
            u3 = tpool.tile([128, NP], EW, tag="u")
            nc.vector.tensor_tensor(u3, u2, d[2], OP.max)
            v1 = tpool.tile([128, NP], EW, tag="v")
            nc.vector.tensor_tensor(v1, s_b[:, 0, :], s_b[:, 1, :], OP.min)
            v2 = tpool.tile([128, NP], EW, tag="v")
            nc.vector.scalar_tensor_tensor(
                out=v2, in0=s_b[:, 2, :], scalar=0.0, in1=v1,
                op0=OP.add, op1=OP.min,
            )
            q_t = tpool.tile([128, NP], EW, tag="u")
            nc.vector.scalar_tensor_tensor(
                out=q_t, in0=u3, scalar=-1.0, in1=v2, op0=OP.mult, op1=OP.min
            )
            rsum = smpool.tile([128, 1], F32, tag="rsum")
            r_t = tpool.tile([128, NP], EW, tag="u")
            nc.vector.scalar_tensor_tensor(
                out=r_t, in0=q_t, scalar=0.0, in1=z0,
                op0=OP.max, op1=OP.add, accum_out=rsum,
            )

            o = []
            for w in range(3):
                o_t = opool.tile([128, NP], EW, tag="o")
                nc.vector.scalar_tensor_tensor(
                    out=o_t,
                    in0=d[w],
                    scalar=0.0,
                    in1=s_b[:, w, :],
                    op0=OP.is_gt,
                    op1=OP.mult,
                )
                o.append(o_t)
            t1 = tpool.tile([128, NP], EW, tag="v")
            nc.vector.tensor_tensor(t1, o[0], o[1], OP.max)
            osum = smpool.tile([128, 1], F32, tag="osum")
            ot = tpool.tile([128, NP], EW, tag="v")
            nc.vector.scalar_tensor_tensor(
                out=ot,
                in0=t1,
                scalar=0.0,
                in1=o[2],
                op0=OP.add,
                op1=OP.max,
                accum_out=osum,
            )

            minn_s = smpool.tile([128, 1], F32, tag="minn")
            nc.vector.tensor_add(minn_s, rsum, osum)
            nc.vector.tensor_scalar_mul(minn_s, minn_s, 1.0 / NP)
            nc.sync.dma_start(minn_d[bsl, :], minn_s)

    return nc


def _run(inputs, trace=False, n_rows=BZC, n_cores=NCORES, trace_kwargs=None):
    """Shard inputs, build + run the SPMD program, gather outputs.

    Returns (outputs_tuple, BassKernelResults).
    """
    f = lambda a: np.asarray(a, dtype=np.float32)
    x = f(inputs["x"])
    bz = x.shape[0]
    assert bz == n_rows * n_cores
    names = [
        "W1", "b1", "W2", "b2", "W3", "b3", "cW1", "cb1", "cW2", "cb2", "cW3",
        "cb3",
    ]
    ws = {k: f(inputs[k]) for k in names}
    ws["b3"] = ws["b3"].reshape(1, NF)
    ws["cb3"] = ws["cb3"].reshape(1, 3)
    with_b3 = bool(np.any(ws["b3"]))
    with_cb3 = bool(np.any(ws["cb3"]))

    nc = _build(with_b3, with_cb3, n_rows=n_rows)
    nc.finalize()  # Bacc: runs wait-splitting + regalloc passes

    in_maps = []
    for i in range(n_cores):
        xs = x[i * n_rows : (i + 1) * n_rows]
        m = {"xT": np.ascontiguousarray(xs.T)}
        m.update(ws)
        in_maps.append(m)

    res = run_bass_kernel_spmd(
        nc,
        in_maps,
        list(range(n_cores)),
        trace=trace,
        **(trace_kwargs or {}),
    )
    pts = np.concatenate([r["pts"] for r in res.results], axis=0)
    minn = np.concatenate(
        [r["minn"][:, 0] for r in res.results], axis=0
    )
    cub = np.concatenate([r["cub"] for r in res.results], axis=0)
    eye = np.broadcast_to(np.eye(3, dtype=np.float32), (bz, 3, 3)).copy()
    return (pts, minn, cub, eye), res


def kernel(**inputs):
    out, _ = _run(inputs, trace=False)
    return out


# revision 15
# speedup vs baseline: 3.6979x; 3.6979x over previous
"""Trainium2 Bass kernel for nn_DecoderFCWithCuboic.

Data-parallel over 8 NeuronCores: batch 4096 -> 512 rows/core, MLP weights
replicated. Per core:
  points branch: h1 = lrelu(x@W1+b1); h2 = lrelu(h1@W2+b2);
                 pts = sigmoid(h2@W3+b3)            (512, 6144)
  cuboid branch: cub = sigmoid(lrelu(lrelu(x@cW1+cb1)@cW2+cb2)@cW3+cb3)
  cuboid distance (per batch row b, point p):
     d'_w = c_w - s_w            (sign = inlier mask; exact under bf16)
     minn = mean_p( relu(min(min3(d'), min3(s))) + relu(max3(min(s, -BIG*d'))) )
  identities used:
     min3_w min(s, c-s) = min(min3(s), min3(c-s))
     s*[s>c] == relu(min(s, -BIG*(c-s)))   elementwise (BIG=1e30; any
       nonzero f32 d' has |d'|>=1e-8 here so BIG*|d'| >> 1, and d'==0
       gives 0 == the reference's inlier value)
     max3(s*[s>c]) is 0 when all coords are inliers, so the two terms
       just add -- no inlier-indicator select is needed.

The pts matmul runs in bf16 (W3 and h2 cast); everything else f32. The
cuboid-distance vector work runs in bf16 on DVE; d' is produced by the
Scalar engine (ACT) with the per-partition cub bias fused.
"""

import os
import sys

import numpy as np

for _p in ("/opt/trn_rl_repo", "/root/.axon_site/_ro/trn_rl_repo"):
    if os.path.isdir(_p) and _p not in sys.path:
        sys.path.insert(0, _p)

import ml_dtypes
import concourse.bass as bass  # noqa: F401
import concourse.mybir as mybir
import concourse.tile as tile
from concourse import bacc
from concourse.bass import ds, ts
from concourse.bass_utils import run_bass_kernel_spmd

BZ, D, H, NP = 4096, 128, 256, 2048  # batch, in-dim, hidden, points
NCORES = 8
BZC = BZ // NCORES  # 512 batch rows per core
NF = 3 * NP  # 6144 point features
F32 = mybir.dt.float32
BF = mybir.dt.bfloat16
EW = BF  # dtype for cuboid-distance intermediates
AF = mybir.ActivationFunctionType
OP = mybir.AluOpType
SLOPE = 0.01  # torch LeakyReLU default
BIG = 1.0e30


def _lrelu(nc, pool, z_psum, out_sbuf, bias_col):
    """out = lrelu(z + bias). ACT moves PSUM->SBUF with the per-partition bias
    folded in; DVE then applies max(x, SLOPE*x) in one fused op."""
    zsb = pool.tile(list(z_psum.shape), F32, tag="zsb")
    nc.scalar.activation(zsb, z_psum, AF.Identity, bias=bias_col, scale=1.0)
    nc.vector.scalar_tensor_tensor(
        out=out_sbuf, in0=zsb, scalar=SLOPE, in1=zsb, op0=OP.mult, op1=OP.max
    )


def _build(with_b3: bool, with_cb3: bool, n_rows: int = BZC):
    """Build the single-core SPMD program. n_rows must be a multiple of 128."""
    nc = bacc.Bacc()
    nbt = n_rows // 128  # batch sub-tiles

    xT_d = nc.dram_tensor("xT", [D, n_rows], F32, kind="ExternalInput")
    W1_d = nc.dram_tensor("W1", [D, H], F32, kind="ExternalInput")
    b1_d = nc.dram_tensor("b1", [H], F32, kind="ExternalInput")
    W2_d = nc.dram_tensor("W2", [H, H], F32, kind="ExternalInput")
    b2_d = nc.dram_tensor("b2", [H], F32, kind="ExternalInput")
    W3_d = nc.dram_tensor("W3", [H, NF], BF, kind="ExternalInput")
    b3_d = nc.dram_tensor("b3", [1, NF], BF, kind="ExternalInput")
    cW1_d = nc.dram_tensor("cW1", [D, H], F32, kind="ExternalInput")
    cb1_d = nc.dram_tensor("cb1", [H], F32, kind="ExternalInput")
    cW2_d = nc.dram_tensor("cW2", [H, H], F32, kind="ExternalInput")
    cb2_d = nc.dram_tensor("cb2", [H], F32, kind="ExternalInput")
    cW3_d = nc.dram_tensor("cW3", [H, 3], F32, kind="ExternalInput")
    cb3_d = nc.dram_tensor("cb3", [1, 3], F32, kind="ExternalInput")

    pts_d = nc.dram_tensor("pts", [n_rows, 3, NP], F32, kind="ExternalOutput")
    minn_d = nc.dram_tensor("minn", [n_rows, 1], F32, kind="ExternalOutput")
    cub_d = nc.dram_tensor("cub", [n_rows, 3], F32, kind="ExternalOutput")

    from contextlib import ExitStack

    with tile.TileContext(nc) as tc, ExitStack() as ctx:
        consts = ctx.enter_context(tc.tile_pool(name="consts", bufs=1))

        # ---- resident weights / biases -------------------------------------
        xT = consts.tile([D, n_rows], F32)
        nc.sync.dma_start(xT, xT_d[:, :])
        W3 = consts.tile([128, 2, NF], BF)
        nc.sync.dma_start(W3, W3_d[:, :].rearrange("(k p) n -> p k n", p=128))
        W1 = consts.tile([128, H], F32)
        nc.sync.dma_start(W1, W1_d[:, :])
        W2 = consts.tile([128, 2, H], F32)
        nc.sync.dma_start(W2, W2_d[:, :].rearrange("(k p) m -> p k m", p=128))
        cW1 = consts.tile([128, H], F32)
        nc.sync.dma_start(cW1, cW1_d[:, :])
        cW2 = consts.tile([128, 2, H], F32)
        nc.sync.dma_start(cW2, cW2_d[:, :].rearrange("(k p) m -> p k m", p=128))
        cW3 = consts.tile([128, 2, 3], F32)
        nc.sync.dma_start(cW3, cW3_d[:, :].rearrange("(k p) n -> p k n", p=128))
        b1 = consts.tile([128, 2], F32)
        nc.sync.dma_start(b1, b1_d[:].rearrange("(m p) -> p m", p=128))
        b2 = consts.tile([128, 2], F32)
        nc.sync.dma_start(b2, b2_d[:].rearrange("(m p) -> p m", p=128))
        cb1 = consts.tile([128, 2], F32)
        nc.sync.dma_start(cb1, cb1_d[:].rearrange("(m p) -> p m", p=128))
        cb2 = consts.tile([128, 2], F32)
        nc.sync.dma_start(cb2, cb2_d[:].rearrange("(m p) -> p m", p=128))
        cb3 = consts.tile([1, 3], F32)
        nc.sync.dma_start(cb3, cb3_d[:, :])
        ones = consts.tile([1, 128], F32)
        nc.vector.memset(ones, 1.0)
        b3 = None
        onesb = None
        if with_b3:
            b3 = consts.tile([1, NF], BF)
            nc.sync.dma_start(b3, b3_d[:, :])
            onesb = consts.tile([1, 128], BF)
            nc.vector.memset(onesb, 1.0)

        # ---- layer phase: h1/h2/c1/c2, feature-major [feat, batch] ---------
        hpool = ctx.enter_context(tc.tile_pool(name="hpool", bufs=1))
        h2 = hpool.tile([128, 2, n_rows], BF)
        c2 = hpool.tile([128, 2, n_rows], F32)
        h1 = hpool.tile([128, 2, n_rows], F32)
        c1 = hpool.tile([128, 2, n_rows], F32)
        with (
            tc.tile_pool(name="lay_psum", bufs=4, space="PSUM") as lpsum,
            tc.tile_pool(name="zpool", bufs=2) as zpool,
        ):
            for W_in, b_in, out_t in ((W1, b1, h1), (cW1, cb1, c1)):
                for m in range(2):
                    zp = lpsum.tile([128, n_rows], F32, tag="zp")
                    nc.tensor.matmul(
                        zp, W_in[:, ts(m, 128)], xT[:, :], start=True, stop=True
                    )
                    _lrelu(nc, zpool, zp, out_t[:, m, :], b_in[:, m : m + 1])
            for W_in, b_in, in_t, out_t in ((W2, b2, h1, h2), (cW2, cb2, c1, c2)):
                for m in range(2):
                    zp = lpsum.tile([128, n_rows], F32, tag="zp")
                    for k in range(2):
                        nc.tensor.matmul(
                            zp,
                            W_in[:, k, ts(m, 128)],
                            in_t[:, k, :],
                            start=(k == 0),
                            stop=(k == 1),
                        )
                    _lrelu(nc, zpool, zp, out_t[:, m, :], b_in[:, m : m + 1])

        # ---- per-sub-tile phase --------------------------------------------
        spool = ctx.enter_context(tc.tile_pool(name="spool", bufs=2))
        sbpool = ctx.enter_context(tc.tile_pool(name="sbpool", bufs=2))
        dpool = ctx.enter_context(tc.tile_pool(name="dpool", bufs=2))
        epool = ctx.enter_context(tc.tile_pool(name="epool", bufs=1))
        opool = ctx.enter_context(tc.tile_pool(name="opool", bufs=2))
        tpool = ctx.enter_context(tc.tile_pool(name="tpool", bufs=2))
        cpool = ctx.enter_context(tc.tile_pool(name="cpool", bufs=2))
        smpool = ctx.enter_context(tc.tile_pool(name="smpool", bufs=4))
        pts_psum = ctx.enter_context(
            tc.tile_pool(name="pts_psum", bufs=2, space="PSUM")
        )

        for bt in range(nbt):
            bsl = ts(bt, 128)
            # cuboid head: cub[128, 3] (batch-partition layout); borrows a
            # pts_psum slot.
            cp = pts_psum.tile([128, NP], F32, tag="pp")
            if with_cb3:
                nc.tensor.matmul(cp[:, 0:3], ones, cb3, start=True, stop=False)
            for k in range(2):
                nc.tensor.matmul(
                    cp[:, 0:3],
                    c2[:, k, bsl],
                    cW3[:, k, :],
                    start=(k == 0 and not with_cb3),
                    stop=(k == 1),
                )
            cub_s = cpool.tile([128, 3], F32, tag="cub")
            nc.scalar.activation(cub_s, cp[:, 0:3], AF.Sigmoid)
            nc.sync.dma_start(cub_d[bsl, :], cub_s)

            # pts matmuls (bf16) + sigmoid: s [128, 3, 2048] f32
            s = spool.tile([128, 3, NP], F32, tag="s")
            for w in range(3):
                pp = pts_psum.tile([128, NP], F32, tag="pp")
                if with_b3:
                    for nn in range(4):
                        nc.tensor.matmul(
                            pp[:, ts(nn, 512)],
                            onesb,
                            b3[:, ds(w * NP + nn * 512, 512)],
                            start=True,
                            stop=False,
                        )
                for k in range(2):
                    for nn in range(4):
                        nc.tensor.matmul(
                            pp[:, ts(nn, 512)],
                            h2[:, k, bsl],
                            W3[:, k, ds(w * NP + nn * 512, 512)],
                            start=(k == 0 and not with_b3),
                            stop=(k == 1),
                        )
                nc.scalar.activation(s[:, w, :], pp, AF.Sigmoid)
            nc.sync.dma_start(pts_d[bsl, :, :], s)

            # bf16 shadow of s (one slice on GPSIMD, two on DVE)
            s_b = sbpool.tile([128, 3, NP], EW, tag="sb")
            nc.gpsimd.tensor_copy(out=s_b[:, 0, :], in_=s[:, 0, :])
            nc.vector.tensor_copy(out=s_b[:, 1, :], in_=s[:, 1, :])
            nc.vector.tensor_copy(out=s_b[:, 2, :], in_=s[:, 2, :])

            # d'_w = c_w - s_w on ACT (bias=cub, scale=-1), bf16 out
            dp = dpool.tile([128, 3, NP], EW, tag="d")
            for w in range(3):
                nc.scalar.activation(
                    dp[:, w, :],
                    s[:, w, :],
                    AF.Identity,
                    bias=cub_s[:, w : w + 1],
                    scale=-1.0,
                )

            # O-side: o = min(s, -BIG*d'), Osum = sum relu(max3(o))
            eh = epool.tile([128, 3, NP], EW, tag="e")
            nc.vector.tensor_scalar(
                out=eh, in0=dp, scalar1=-BIG, scalar2=None, op0=OP.mult
            )
            o_t = opool.tile([128, 3, NP], EW, tag="o")
            nc.vector.tensor_tensor(o_t, s_b, eh, OP.min)
            t1 = tpool.tile([128, NP], EW, tag="t")
            nc.vector.tensor_tensor(t1, o_t[:, 0, :], o_t[:, 1, :], OP.max)
            t2 = tpool.tile([128, NP], EW, tag="t")
            nc.vector.tensor_tensor(t2, t1, o_t[:, 2, :], OP.max)
            osum = smpool.tile([128, 1], F32, tag="osum")
            t3 = tpool.tile([128, NP], EW, tag="t")
            nc.vector.tensor_scalar(
                out=t3, in0=t2, scalar1=0.0, scalar2=0.0,
                op0=OP.max, op1=OP.add, accum_out=osum,
            )

            # R-side: Rsum = sum relu(min(min3(d'), min3(s)))
            u1 = tpool.tile([128, NP], EW, tag="u")
            nc.vector.tensor_tensor(u1, dp[:, 0, :], dp[:, 1, :], OP.min)
            u2 = tpool.tile([128, NP], EW, tag="u")
            nc.vector.tensor_tensor(u2, u1, dp[:, 2, :], OP.min)
            v1 = tpool.tile([128, NP], EW, tag="v")
            nc.vector.tensor_tensor(v1, s_b[:, 0, :], s_b[:, 1, :], OP.min)
            v2 = tpool.tile([128, NP], EW, tag="v")
            nc.vector.tensor_tensor(v2, v1, s_b[:, 2, :], OP.min)
            q1 = tpool.tile([128, NP], EW, tag="u")
            nc.vector.tensor_tensor(q1, u2, v2, OP.min)
            rsum = smpool.tile([128, 1], F32, tag="rsum")
            q2 = tpool.tile([128, NP], EW, tag="v")
            nc.vector.tensor_scalar(
                out=q2, in0=q1, scalar1=0.0, scalar2=0.0,
                op0=OP.max, op1=OP.add, accum_out=rsum,
            )

            minn_s = smpool.tile([128, 1], F32, tag="minn")
            nc.vector.tensor_add(minn_s, rsum, osum)
            nc.vector.tensor_scalar_mul(minn_s, minn_s, 1.0 / NP)
            nc.sync.dma_start(minn_d[bsl, :], minn_s)

    return nc


def _run(inputs, trace=False, n_rows=BZC, n_cores=NCORES, trace_kwargs=None):
    """Shard inputs, build + run the SPMD program, gather outputs.

    Returns (outputs_tuple, BassKernelResults).
    """
    f = lambda a: np.asarray(a, dtype=np.float32)
    x = f(inputs["x"])
    bz = x.shape[0]
    assert bz == n_rows * n_cores
    names = [
        "W1", "b1", "W2", "b2", "W3", "b3", "cW1", "cb1", "cW2", "cb2", "cW3",
        "cb3",
    ]
    ws = {k: f(inputs[k]) for k in names}
    ws["W3"] = ws["W3"].astype(ml_dtypes.bfloat16)
    ws["b3"] = ws["b3"].reshape(1, NF).astype(ml_dtypes.bfloat16)
    ws["cb3"] = ws["cb3"].reshape(1, 3)
    with_b3 = bool(np.any(inputs["b3"]))
    with_cb3 = bool(np.any(ws["cb3"]))

    nc = _build(with_b3, with_cb3, n_rows=n_rows)
    nc.finalize()  # Bacc: runs wait-splitting + regalloc passes

    in_maps = []
    for i in range(n_cores):
        xs = x[i * n_rows : (i + 1) * n_rows]
        m = {"xT": np.ascontiguousarray(xs.T)}
        m.update(ws)
        in_maps.append(m)

    res = run_bass_kernel_spmd(
        nc,
        in_maps,
        list(range(n_cores)),
        trace=trace,
        **(trace_kwargs or {}),
    )
    pts = np.concatenate([r["pts"] for r in res.results], axis=0)
    minn = np.concatenate(
        [r["minn"][:, 0] for r in res.results], axis=0
    )
    cub = np.concatenate([r["cub"] for r in res.results], axis=0)
    eye = np.broadcast_to(np.eye(3, dtype=np.float32), (bz, 3, 3)).copy()
    return (pts, minn, cub, eye), res


def kernel(**inputs):
    out, _ = _run(inputs, trace=False)
    return out


# revision 18
# speedup vs baseline: 3.8273x; 1.0350x over previous
"""Trainium2 Bass kernel for nn_DecoderFCWithCuboic.

Data-parallel over 8 NeuronCores: batch 4096 -> 512 rows/core, MLP weights
replicated. Per core:
  points branch: h1 = lrelu(x@W1+b1); h2 = lrelu(h1@W2+b2);
                 pts = sigmoid(h2@W3+b3)            (512, 6144)
  cuboid branch: cub = sigmoid(lrelu(lrelu(x@cW1+cb1)@cW2+cb2)@cW3+cb3)
  cuboid distance (per batch row b, point p):
     d'_w = c_w - s_w            (sign = inlier mask; exact under bf16)
     minn = mean_p( relu(min(min3(d'), min3(s))) + relu(max3(min(s, -BIG*d'))) )
  identities used:
     min3_w min(s, c-s) = min(min3(s), min3(c-s))
     s*[s>c] == relu(min(s, -BIG*(c-s)))   elementwise (BIG=1e30; any
       nonzero f32 d' has |d'|>=1e-8 here so BIG*|d'| >> 1, and d'==0
       gives 0 == the reference's inlier value)
     max3(s*[s>c]) is 0 when all coords are inliers, so the two terms
       just add -- no inlier-indicator select is needed.

The pts matmul runs in bf16 (W3 and h2 cast); everything else f32. The
cuboid-distance vector work runs in bf16 on DVE; d' is produced by the
Scalar engine (ACT) with the per-partition cub bias fused.
"""

import os
import sys

import numpy as np

for _p in ("/opt/trn_rl_repo", "/root/.axon_site/_ro/trn_rl_repo"):
    if os.path.isdir(_p) and _p not in sys.path:
        sys.path.insert(0, _p)

import ml_dtypes
import concourse.bass as bass  # noqa: F401
import concourse.mybir as mybir
import concourse.tile as tile
from concourse import bacc
from concourse.bass import ds, ts
from concourse.bass_utils import run_bass_kernel_spmd

BZ, D, H, NP = 4096, 128, 256, 2048  # batch, in-dim, hidden, points
NCORES = 8
BZC = BZ // NCORES  # 512 batch rows per core
NF = 3 * NP  # 6144 point features
F32 = mybir.dt.float32
BF = mybir.dt.bfloat16
EW = BF  # dtype for cuboid-distance intermediates
AF = mybir.ActivationFunctionType
OP = mybir.AluOpType
SLOPE = 0.01  # torch LeakyReLU default
BIG = 1.0e30


def _lrelu(nc, pool, z_psum, out_sbuf, bias_col):
    """out = lrelu(z + bias). ACT moves PSUM->SBUF with the per-partition bias
    folded in; DVE then applies max(x, SLOPE*x) in one fused op."""
    zsb = pool.tile(list(z_psum.shape), F32, tag="zsb")
    nc.scalar.activation(zsb, z_psum, AF.Identity, bias=bias_col, scale=1.0)
    nc.vector.scalar_tensor_tensor(
        out=out_sbuf, in0=zsb, scalar=SLOPE, in1=zsb, op0=OP.mult, op1=OP.max
    )


def _build(with_b3: bool, with_cb3: bool, n_rows: int = BZC):
    """Build the single-core SPMD program. n_rows must be a multiple of 128."""
    nc = bacc.Bacc()
    nbt = n_rows // 128  # batch sub-tiles

    xT_d = nc.dram_tensor("xT", [D, n_rows], F32, kind="ExternalInput")
    W1_d = nc.dram_tensor("W1", [D, H], F32, kind="ExternalInput")
    b1_d = nc.dram_tensor("b1", [H], F32, kind="ExternalInput")
    W2_d = nc.dram_tensor("W2", [H, H], F32, kind="ExternalInput")
    b2_d = nc.dram_tensor("b2", [H], F32, kind="ExternalInput")
    W3_d = nc.dram_tensor("W3", [H, NF], BF, kind="ExternalInput")
    b3_d = nc.dram_tensor("b3", [1, NF], BF, kind="ExternalInput")
    cW1_d = nc.dram_tensor("cW1", [D, H], F32, kind="ExternalInput")
    cb1_d = nc.dram_tensor("cb1", [H], F32, kind="ExternalInput")
    cW2_d = nc.dram_tensor("cW2", [H, H], F32, kind="ExternalInput")
    cb2_d = nc.dram_tensor("cb2", [H], F32, kind="ExternalInput")
    cW3_d = nc.dram_tensor("cW3", [H, 3], F32, kind="ExternalInput")
    cb3_d = nc.dram_tensor("cb3", [1, 3], F32, kind="ExternalInput")

    pts_d = nc.dram_tensor("pts", [n_rows, 3, NP], F32, kind="ExternalOutput")
    minn_d = nc.dram_tensor("minn", [n_rows, 1], F32, kind="ExternalOutput")
    cub_d = nc.dram_tensor("cub", [n_rows, 3], F32, kind="ExternalOutput")

    from contextlib import ExitStack

    with tile.TileContext(nc) as tc, ExitStack() as ctx:
        consts = ctx.enter_context(tc.tile_pool(name="consts", bufs=1))

        # ---- resident weights / biases -------------------------------------
        xT = consts.tile([D, n_rows], F32)
        nc.sync.dma_start(xT, xT_d[:, :])
        W3 = consts.tile([128, 2, NF], BF)
        nc.sync.dma_start(W3, W3_d[:, :].rearrange("(k p) n -> p k n", p=128))
        W1 = consts.tile([128, H], F32)
        nc.sync.dma_start(W1, W1_d[:, :])
        W2 = consts.tile([128, 2, H], F32)
        nc.sync.dma_start(W2, W2_d[:, :].rearrange("(k p) m -> p k m", p=128))
        cW1 = consts.tile([128, H], F32)
        nc.sync.dma_start(cW1, cW1_d[:, :])
        cW2 = consts.tile([128, 2, H], F32)
        nc.sync.dma_start(cW2, cW2_d[:, :].rearrange("(k p) m -> p k m", p=128))
        cW3 = consts.tile([128, 2, 3], F32)
        nc.sync.dma_start(cW3, cW3_d[:, :].rearrange("(k p) n -> p k n", p=128))
        b1 = consts.tile([128, 2], F32)
        nc.sync.dma_start(b1, b1_d[:].rearrange("(m p) -> p m", p=128))
        b2 = consts.tile([128, 2], F32)
        nc.sync.dma_start(b2, b2_d[:].rearrange("(m p) -> p m", p=128))
        cb1 = consts.tile([128, 2], F32)
        nc.sync.dma_start(cb1, cb1_d[:].rearrange("(m p) -> p m", p=128))
        cb2 = consts.tile([128, 2], F32)
        nc.sync.dma_start(cb2, cb2_d[:].rearrange("(m p) -> p m", p=128))
        cb3 = consts.tile([1, 3], F32)
        nc.sync.dma_start(cb3, cb3_d[:, :])
        ones = consts.tile([1, 128], F32)
        nc.vector.memset(ones, 1.0)
        b3 = None
        onesb = None
        if with_b3:
            b3 = consts.tile([1, NF], BF)
            nc.sync.dma_start(b3, b3_d[:, :])
            onesb = consts.tile([1, 128], BF)
            nc.vector.memset(onesb, 1.0)

        # ---- layer phase: h1/h2/c1/c2, feature-major [feat, batch] ---------
        hpool = ctx.enter_context(tc.tile_pool(name="hpool", bufs=1))
        h2 = hpool.tile([128, 2, n_rows], BF)
        c2 = hpool.tile([128, 2, n_rows], F32)
        with (
            tc.tile_pool(name="lay_psum", bufs=4, space="PSUM") as lpsum,
            tc.tile_pool(name="zpool", bufs=2) as zpool,
            tc.tile_pool(name="h1pool", bufs=1) as h1pool,
        ):
            h1 = h1pool.tile([128, 2, n_rows], F32)
            c1 = h1pool.tile([128, 2, n_rows], F32)
            for W_in, b_in, out_t in ((W1, b1, h1), (cW1, cb1, c1)):
                for m in range(2):
                    zp = lpsum.tile([128, n_rows], F32, tag="zp")
                    nc.tensor.matmul(
                        zp, W_in[:, ts(m, 128)], xT[:, :], start=True, stop=True
                    )
                    _lrelu(nc, zpool, zp, out_t[:, m, :], b_in[:, m : m + 1])
            for W_in, b_in, in_t, out_t in ((W2, b2, h1, h2), (cW2, cb2, c1, c2)):
                for m in range(2):
                    zp = lpsum.tile([128, n_rows], F32, tag="zp")
                    for k in range(2):
                        nc.tensor.matmul(
                            zp,
                            W_in[:, k, ts(m, 128)],
                            in_t[:, k, :],
                            start=(k == 0),
                            stop=(k == 1),
                        )
                    _lrelu(nc, zpool, zp, out_t[:, m, :], b_in[:, m : m + 1])

        # ---- per-sub-tile phase --------------------------------------------
        spool = ctx.enter_context(tc.tile_pool(name="spool", bufs=2))
        sbpool = ctx.enter_context(tc.tile_pool(name="sbpool", bufs=2))
        dpool = ctx.enter_context(tc.tile_pool(name="dpool", bufs=2))
        epool = ctx.enter_context(tc.tile_pool(name="epool", bufs=1))
        opool = ctx.enter_context(tc.tile_pool(name="opool", bufs=2))
        tpool = ctx.enter_context(tc.tile_pool(name="tpool", bufs=3))
        cpool = ctx.enter_context(tc.tile_pool(name="cpool", bufs=2))
        smpool = ctx.enter_context(tc.tile_pool(name="smpool", bufs=4))
        pts_psum = ctx.enter_context(
            tc.tile_pool(name="pts_psum", bufs=2, space="PSUM")
        )

        for bt in range(nbt):
            bsl = ts(bt, 128)
            # cuboid head: cub[128, 3] (batch-partition layout); borrows a
            # pts_psum slot.
            cp = pts_psum.tile([128, NP], F32, tag="pp")
            if with_cb3:
                nc.tensor.matmul(cp[:, 0:3], ones, cb3, start=True, stop=False)
            for k in range(2):
                nc.tensor.matmul(
                    cp[:, 0:3],
                    c2[:, k, bsl],
                    cW3[:, k, :],
                    start=(k == 0 and not with_cb3),
                    stop=(k == 1),
                )
            cub_s = cpool.tile([128, 3], F32, tag="cub")
            nc.scalar.activation(cub_s, cp[:, 0:3], AF.Sigmoid)
            nc.sync.dma_start(cub_d[bsl, :], cub_s)

            # pts matmuls (bf16) + sigmoid: s [128, 3, 2048] f32
            s = spool.tile([128, 3, NP], F32, tag="s")
            for w in range(3):
                pp = pts_psum.tile([128, NP], F32, tag="pp")
                if with_b3:
                    for nn in range(4):
                        nc.tensor.matmul(
                            pp[:, ts(nn, 512)],
                            onesb,
                            b3[:, ds(w * NP + nn * 512, 512)],
                            start=True,
                            stop=False,
                        )
                for k in range(2):
                    for nn in range(4):
                        nc.tensor.matmul(
                            pp[:, ts(nn, 512)],
                            h2[:, k, bsl],
                            W3[:, k, ds(w * NP + nn * 512, 512)],
                            start=(k == 0 and not with_b3),
                            stop=(k == 1),
                        )
                nc.scalar.activation(s[:, w, :], pp, AF.Sigmoid)
            nc.sync.dma_start(pts_d[bsl, :, :], s)

            # bf16 shadow of s (one slice on GPSIMD, two on DVE)
            s_b = sbpool.tile([128, 3, NP], EW, tag="sb")
            nc.gpsimd.tensor_copy(out=s_b[:, 0, :], in_=s[:, 0, :])
            nc.vector.tensor_copy(out=s_b[:, 1, :], in_=s[:, 1, :])
            nc.vector.tensor_copy(out=s_b[:, 2, :], in_=s[:, 2, :])

            # d'_w = c_w - s_w on ACT (bias=cub, scale=-1), bf16 out
            dp = dpool.tile([128, 3, NP], EW, tag="d")
            for w in range(3):
                nc.scalar.activation(
                    dp[:, w, :],
                    s[:, w, :],
                    AF.Identity,
                    bias=cub_s[:, w : w + 1],
                    scale=-1.0,
                )

            # O-side: o = min(s, -BIG*d'), Osum = sum relu(max3(o))
            eh = epool.tile([128, 3, NP], EW, tag="e")
            nc.vector.tensor_scalar(
                out=eh, in0=dp, scalar1=-BIG, scalar2=None, op0=OP.mult
            )
            o_t = opool.tile([128, 3, NP], EW, tag="o")
            nc.vector.tensor_tensor(o_t, s_b, eh, OP.min)
            t1 = tpool.tile([128, NP], EW, tag="t")
            nc.vector.tensor_tensor(t1, o_t[:, 0, :], o_t[:, 1, :], OP.max)
            t2 = tpool.tile([128, NP], EW, tag="t")
            nc.vector.tensor_tensor(t2, t1, o_t[:, 2, :], OP.max)

            # R-side: Rterm = min(min3(d'), min3(s)).  Rterm > 0 only when all
            # coords are inliers, and then the O-side max3 is <= 0 -- the two
            # terms are mutually exclusive, so one fused relu+sum suffices:
            #   result = relu(max(Rterm, Oterm))
            u1 = tpool.tile([128, NP], EW, tag="u")
            nc.vector.tensor_tensor(u1, dp[:, 0, :], dp[:, 1, :], OP.min)
            u2 = tpool.tile([128, NP], EW, tag="u")
            nc.vector.tensor_tensor(u2, u1, dp[:, 2, :], OP.min)
            v1 = tpool.tile([128, NP], EW, tag="v")
            nc.vector.tensor_tensor(v1, s_b[:, 0, :], s_b[:, 1, :], OP.min)
            v2 = tpool.tile([128, NP], EW, tag="v")
            nc.vector.tensor_tensor(v2, v1, s_b[:, 2, :], OP.min)
            q1 = tpool.tile([128, NP], EW, tag="u")
            nc.vector.tensor_tensor(q1, u2, v2, OP.min)
            fin = tpool.tile([128, NP], EW, tag="v")
            nc.vector.tensor_tensor(fin, q1, t2, OP.max)
            msum = smpool.tile([128, 1], F32, tag="msum")
            f2 = tpool.tile([128, NP], EW, tag="t")
            nc.vector.tensor_scalar(
                out=f2, in0=fin, scalar1=0.0, scalar2=0.0,
                op0=OP.max, op1=OP.add, accum_out=msum,
            )

            minn_s = smpool.tile([128, 1], F32, tag="minn")
            nc.vector.tensor_scalar_mul(minn_s, msum, 1.0 / NP)
            nc.sync.dma_start(minn_d[bsl, :], minn_s)

    return nc


def _run(inputs, trace=False, n_rows=BZC, n_cores=NCORES, trace_kwargs=None):
    """Shard inputs, build + run the SPMD program, gather outputs.

    Returns (outputs_tuple, BassKernelResults).
    """
    f = lambda a: np.asarray(a, dtype=np.float32)
    x = f(inputs["x"])
    bz = x.shape[0]
    assert bz == n_rows * n_cores
    names = [
        "W1", "b1", "W2", "b2", "W3", "b3", "cW1", "cb1", "cW2", "cb2", "cW3",
        "cb3",
    ]
    ws = {k: f(inputs[k]) for k in names}
    ws["W3"] = ws["W3"].astype(ml_dtypes.bfloat16)
    ws["b3"] = ws["b3"].reshape(1, NF).astype(ml_dtypes.bfloat16)
    ws["cb3"] = ws["cb3"].reshape(1, 3)
    with_b3 = bool(np.any(inputs["b3"]))
    with_cb3 = bool(np.any(ws["cb3"]))

    nc = _build(with_b3, with_cb3, n_rows=n_rows)
    nc.finalize()  # Bacc: runs wait-splitting + regalloc passes

    in_maps = []
    for i in range(n_cores):
        xs = x[i * n_rows : (i + 1) * n_rows]
        m = {"xT": np.ascontiguousarray(xs.T)}
        m.update(ws)
        in_maps.append(m)

    res = run_bass_kernel_spmd(
        nc,
        in_maps,
        list(range(n_cores)),
        trace=trace,
        **(trace_kwargs or {}),
    )
    pts = np.concatenate([r["pts"] for r in res.results], axis=0)
    minn = np.concatenate(
        [r["minn"][:, 0] for r in res.results], axis=0
    )
    cub = np.concatenate([r["cub"] for r in res.results], axis=0)
    eye = np.broadcast_to(np.eye(3, dtype=np.float32), (bz, 3, 3)).copy()
    return (pts, minn, cub, eye), res


def kernel(**inputs):
    out, _ = _run(inputs, trace=False)
    return out


# revision 19
# speedup vs baseline: 4.1995x; 1.0973x over previous
"""Trainium2 Bass kernel for nn_DecoderFCWithCuboic.

Data-parallel over 8 NeuronCores: batch 4096 -> 512 rows/core, MLP weights
replicated. Per core:
  points branch: h1 = lrelu(x@W1+b1); h2 = lrelu(h1@W2+b2);
                 pts = sigmoid(h2@W3+b3)            (512, 6144)
  cuboid branch: cub = sigmoid(lrelu(lrelu(x@cW1+cb1)@cW2+cb2)@cW3+cb3)
  cuboid distance (per batch row b, point p):
     d'_w = c_w - s_w            (sign = inlier mask; exact under bf16)
     minn = mean_p( relu(min(min3(d'), min3(s))) + relu(max3(min(s, -BIG*d'))) )
  identities used:
     min3_w min(s, c-s) = min(min3(s), min3(c-s))
     s*[s>c] == relu(min(s, -BIG*(c-s)))   elementwise (BIG=1e30; any
       nonzero f32 d' has |d'|>=1e-8 here so BIG*|d'| >> 1, and d'==0
       gives 0 == the reference's inlier value)
     max3(s*[s>c]) is 0 when all coords are inliers, so the two terms
       just add -- no inlier-indicator select is needed.

The pts matmul runs in bf16 (W3 and h2 cast); everything else f32. The
cuboid-distance vector work runs in bf16 on DVE; d' is produced by the
Scalar engine (ACT) with the per-partition cub bias fused.
"""

import os
import sys

import numpy as np

for _p in ("/opt/trn_rl_repo", "/root/.axon_site/_ro/trn_rl_repo"):
    if os.path.isdir(_p) and _p not in sys.path:
        sys.path.insert(0, _p)

import ml_dtypes
import concourse.bass as bass  # noqa: F401
import concourse.mybir as mybir
import concourse.tile as tile
from concourse import bacc
from concourse.bass import ds, ts
from concourse.bass_utils import run_bass_kernel_spmd

BZ, D, H, NP = 4096, 128, 256, 2048  # batch, in-dim, hidden, points
NCORES = 8
BZC = BZ // NCORES  # 512 batch rows per core
NF = 3 * NP  # 6144 point features
F32 = mybir.dt.float32
BF = mybir.dt.bfloat16
EW = BF  # dtype for cuboid-distance intermediates
AF = mybir.ActivationFunctionType
OP = mybir.AluOpType
SLOPE = 0.01  # torch LeakyReLU default
BIG = 1.0e30


def _lrelu(nc, pool, z_psum, out_sbuf, bias_col):
    """out = lrelu(z + bias). ACT moves PSUM->SBUF with the per-partition bias
    folded in; DVE then applies max(x, SLOPE*x) in one fused op."""
    zsb = pool.tile(list(z_psum.shape), F32, tag="zsb")
    nc.scalar.activation(zsb, z_psum, AF.Identity, bias=bias_col, scale=1.0)
    nc.vector.scalar_tensor_tensor(
        out=out_sbuf, in0=zsb, scalar=SLOPE, in1=zsb, op0=OP.mult, op1=OP.max
    )


def _build(with_b3: bool, with_cb3: bool, n_rows: int = BZC):
    """Build the single-core SPMD program. n_rows must be a multiple of 128."""
    nc = bacc.Bacc()
    nbt = n_rows // 128  # batch sub-tiles

    xT_d = nc.dram_tensor("xT", [D, n_rows], F32, kind="ExternalInput")
    W1_d = nc.dram_tensor("W1", [D, H], F32, kind="ExternalInput")
    b1_d = nc.dram_tensor("b1", [H], F32, kind="ExternalInput")
    W2_d = nc.dram_tensor("W2", [H, H], F32, kind="ExternalInput")
    b2_d = nc.dram_tensor("b2", [H], F32, kind="ExternalInput")
    W3_d = nc.dram_tensor("W3", [H, NF], BF, kind="ExternalInput")
    b3_d = nc.dram_tensor("b3", [1, NF], BF, kind="ExternalInput")
    cW1_d = nc.dram_tensor("cW1", [D, H], F32, kind="ExternalInput")
    cb1_d = nc.dram_tensor("cb1", [H], F32, kind="ExternalInput")
    cW2_d = nc.dram_tensor("cW2", [H, H], F32, kind="ExternalInput")
    cb2_d = nc.dram_tensor("cb2", [H], F32, kind="ExternalInput")
    cW3_d = nc.dram_tensor("cW3", [H, 3], F32, kind="ExternalInput")
    cb3_d = nc.dram_tensor("cb3", [1, 3], F32, kind="ExternalInput")

    pts_d = nc.dram_tensor("pts", [n_rows, 3, NP], F32, kind="ExternalOutput")
    minn_d = nc.dram_tensor("minn", [n_rows, 1], F32, kind="ExternalOutput")
    cub_d = nc.dram_tensor("cub", [n_rows, 3], F32, kind="ExternalOutput")

    from contextlib import ExitStack

    with tile.TileContext(nc) as tc, ExitStack() as ctx:
        consts = ctx.enter_context(tc.tile_pool(name="consts", bufs=1))

        # ---- resident weights / biases -------------------------------------
        xT = consts.tile([D, n_rows], F32)
        nc.sync.dma_start(xT, xT_d[:, :])
        W3 = consts.tile([128, 2, NF], BF)
        nc.sync.dma_start(W3, W3_d[:, :].rearrange("(k p) n -> p k n", p=128))
        W1 = consts.tile([128, H], F32)
        nc.sync.dma_start(W1, W1_d[:, :])
        W2 = consts.tile([128, 2, H], F32)
        nc.sync.dma_start(W2, W2_d[:, :].rearrange("(k p) m -> p k m", p=128))
        cW1 = consts.tile([128, H], F32)
        nc.sync.dma_start(cW1, cW1_d[:, :])
        cW2 = consts.tile([128, 2, H], F32)
        nc.sync.dma_start(cW2, cW2_d[:, :].rearrange("(k p) m -> p k m", p=128))
        cW3 = consts.tile([128, 2, 3], F32)
        nc.sync.dma_start(cW3, cW3_d[:, :].rearrange("(k p) n -> p k n", p=128))
        b1 = consts.tile([128, 2], F32)
        nc.sync.dma_start(b1, b1_d[:].rearrange("(m p) -> p m", p=128))
        b2 = consts.tile([128, 2], F32)
        nc.sync.dma_start(b2, b2_d[:].rearrange("(m p) -> p m", p=128))
        cb1 = consts.tile([128, 2], F32)
        nc.sync.dma_start(cb1, cb1_d[:].rearrange("(m p) -> p m", p=128))
        cb2 = consts.tile([128, 2], F32)
        nc.sync.dma_start(cb2, cb2_d[:].rearrange("(m p) -> p m", p=128))
        cb3 = consts.tile([1, 3], F32)
        nc.sync.dma_start(cb3, cb3_d[:, :])
        ones = consts.tile([1, 128], F32)
        nc.vector.memset(ones, 1.0)
        b3 = None
        onesb = None
        if with_b3:
            b3 = consts.tile([1, NF], BF)
            nc.sync.dma_start(b3, b3_d[:, :])
            onesb = consts.tile([1, 128], BF)
            nc.vector.memset(onesb, 1.0)

        # ---- layer phase: h1/h2/c1/c2, feature-major [feat, batch] ---------
        hpool = ctx.enter_context(tc.tile_pool(name="hpool", bufs=1))
        h2 = hpool.tile([128, 2, n_rows], BF)
        c2 = hpool.tile([128, 2, n_rows], F32)
        with (
            tc.tile_pool(name="lay_psum", bufs=4, space="PSUM") as lpsum,
            tc.tile_pool(name="zpool", bufs=2) as zpool,
            tc.tile_pool(name="h1pool", bufs=1) as h1pool,
        ):
            h1 = h1pool.tile([128, 2, n_rows], F32)
            c1 = h1pool.tile([128, 2, n_rows], F32)
            for W_in, b_in, out_t in ((W1, b1, h1), (cW1, cb1, c1)):
                for m in range(2):
                    zp = lpsum.tile([128, n_rows], F32, tag="zp")
                    nc.tensor.matmul(
                        zp, W_in[:, ts(m, 128)], xT[:, :], start=True, stop=True
                    )
                    _lrelu(nc, zpool, zp, out_t[:, m, :], b_in[:, m : m + 1])
            for W_in, b_in, in_t, out_t in ((W2, b2, h1, h2), (cW2, cb2, c1, c2)):
                for m in range(2):
                    zp = lpsum.tile([128, n_rows], F32, tag="zp")
                    for k in range(2):
                        nc.tensor.matmul(
                            zp,
                            W_in[:, k, ts(m, 128)],
                            in_t[:, k, :],
                            start=(k == 0),
                            stop=(k == 1),
                        )
                    _lrelu(nc, zpool, zp, out_t[:, m, :], b_in[:, m : m + 1])

        # ---- per-sub-tile phase --------------------------------------------
        spool = ctx.enter_context(tc.tile_pool(name="spool", bufs=2))
        sbpool = ctx.enter_context(tc.tile_pool(name="sbpool", bufs=2))
        dpool = ctx.enter_context(tc.tile_pool(name="dpool", bufs=2))
        epool = ctx.enter_context(tc.tile_pool(name="epool", bufs=1))
        opool = ctx.enter_context(tc.tile_pool(name="opool", bufs=2))
        tpool = ctx.enter_context(tc.tile_pool(name="tpool", bufs=3))
        cpool = ctx.enter_context(tc.tile_pool(name="cpool", bufs=2))
        smpool = ctx.enter_context(tc.tile_pool(name="smpool", bufs=4))
        pts_psum = ctx.enter_context(
            tc.tile_pool(name="pts_psum", bufs=2, space="PSUM")
        )

        for bt in range(nbt):
            bsl = ts(bt, 128)
            # cuboid head: cub[128, 3] (batch-partition layout); borrows a
            # pts_psum slot.
            cp = pts_psum.tile([128, NP], F32, tag="pp")
            if with_cb3:
                nc.tensor.matmul(cp[:, 0:3], ones, cb3, start=True, stop=False)
            for k in range(2):
                nc.tensor.matmul(
                    cp[:, 0:3],
                    c2[:, k, bsl],
                    cW3[:, k, :],
                    start=(k == 0 and not with_cb3),
                    stop=(k == 1),
                )
            cub_s = cpool.tile([128, 3], F32, tag="cub")
            nc.scalar.activation(cub_s, cp[:, 0:3], AF.Sigmoid)
            nc.sync.dma_start(cub_d[bsl, :], cub_s)

            # pts matmuls (bf16) + sigmoid: s [128, 3, 2048] f32
            s = spool.tile([128, 3, NP], F32, tag="s")
            for w in range(3):
                pp = pts_psum.tile([128, NP], F32, tag="pp")
                if with_b3:
                    for nn in range(4):
                        nc.tensor.matmul(
                            pp[:, ts(nn, 512)],
                            onesb,
                            b3[:, ds(w * NP + nn * 512, 512)],
                            start=True,
                            stop=False,
                        )
                for k in range(2):
                    for nn in range(4):
                        nc.tensor.matmul(
                            pp[:, ts(nn, 512)],
                            h2[:, k, bsl],
                            W3[:, k, ds(w * NP + nn * 512, 512)],
                            start=(k == 0 and not with_b3),
                            stop=(k == 1),
                        )
                nc.scalar.activation(s[:, w, :], pp, AF.Sigmoid)
            nc.sync.dma_start(pts_d[bsl, :, :], s)

            # bf16 shadow of s (GPSIMD would hold the shared SBUF port and
            # stall DVE tensor_tensor ops, so everything stays on DVE)
            s_b = sbpool.tile([128, 3, NP], EW, tag="sb")
            nc.vector.tensor_copy(out=s_b, in_=s)

            # d'_w = c_w - s_w on ACT (bias=cub, scale=-1), bf16 out
            dp = dpool.tile([128, 3, NP], EW, tag="d")
            for w in range(3):
                nc.scalar.activation(
                    dp[:, w, :],
                    s[:, w, :],
                    AF.Identity,
                    bias=cub_s[:, w : w + 1],
                    scale=-1.0,
                )

            # O-side: o = min(s, -BIG*d'), Osum = sum relu(max3(o))
            eh = epool.tile([128, 3, NP], EW, tag="e")
            nc.vector.tensor_scalar(
                out=eh, in0=dp, scalar1=-BIG, scalar2=None, op0=OP.mult
            )
            o_t = opool.tile([128, 3, NP], EW, tag="o")
            nc.vector.tensor_tensor(o_t, s_b, eh, OP.min)
            t1 = tpool.tile([128, NP], EW, tag="t")
            nc.vector.tensor_tensor(t1, o_t[:, 0, :], o_t[:, 1, :], OP.max)
            t2 = tpool.tile([128, NP], EW, tag="t")
            nc.vector.tensor_tensor(t2, t1, o_t[:, 2, :], OP.max)

            # R-side: Rterm = min(min3(d'), min3(s)).  Rterm > 0 only when all
            # coords are inliers, and then the O-side max3 is <= 0 -- the two
            # terms are mutually exclusive, so one fused relu+sum suffices:
            #   result = relu(max(Rterm, Oterm))
            u1 = tpool.tile([128, NP], EW, tag="u")
            nc.vector.tensor_tensor(u1, dp[:, 0, :], dp[:, 1, :], OP.min)
            u2 = tpool.tile([128, NP], EW, tag="u")
            nc.vector.tensor_tensor(u2, u1, dp[:, 2, :], OP.min)
            v1 = tpool.tile([128, NP], EW, tag="v")
            nc.vector.tensor_tensor(v1, s_b[:, 0, :], s_b[:, 1, :], OP.min)
            v2 = tpool.tile([128, NP], EW, tag="v")
            nc.vector.tensor_tensor(v2, v1, s_b[:, 2, :], OP.min)
            q1 = tpool.tile([128, NP], EW, tag="u")
            nc.vector.tensor_tensor(q1, u2, v2, OP.min)
            fin = tpool.tile([128, NP], EW, tag="v")
            nc.vector.tensor_tensor(fin, q1, t2, OP.max)
            msum = smpool.tile([128, 1], F32, tag="msum")
            f2 = tpool.tile([128, NP], EW, tag="t")
            nc.vector.tensor_scalar(
                out=f2, in0=fin, scalar1=0.0, scalar2=0.0,
                op0=OP.max, op1=OP.add, accum_out=msum,
            )

            minn_s = smpool.tile([128, 1], F32, tag="minn")
            nc.vector.tensor_scalar_mul(minn_s, msum, 1.0 / NP)
            nc.sync.dma_start(minn_d[bsl, :], minn_s)

    return nc


def _run(inputs, trace=False, n_rows=BZC, n_cores=NCORES, trace_kwargs=None):
    """Shard inputs, build + run the SPMD program, gather outputs.

    Returns (outputs_tuple, BassKernelResults).
    """
    f = lambda a: np.asarray(a, dtype=np.float32)
    x = f(inputs["x"])
    bz = x.shape[0]
    assert bz == n_rows * n_cores
    names = [
        "W1", "b1", "W2", "b2", "W3", "b3", "cW1", "cb1", "cW2", "cb2", "cW3",
        "cb3",
    ]
    ws = {k: f(inputs[k]) for k in names}
    ws["W3"] = ws["W3"].astype(ml_dtypes.bfloat16)
    ws["b3"] = ws["b3"].reshape(1, NF).astype(ml_dtypes.bfloat16)
    ws["cb3"] = ws["cb3"].reshape(1, 3)
    with_b3 = bool(np.any(inputs["b3"]))
    with_cb3 = bool(np.any(ws["cb3"]))

    nc = _build(with_b3, with_cb3, n_rows=n_rows)
    nc.finalize()  # Bacc: runs wait-splitting + regalloc passes

    in_maps = []
    for i in range(n_cores):
        xs = x[i * n_rows : (i + 1) * n_rows]
        m = {"xT": np.ascontiguousarray(xs.T)}
        m.update(ws)
        in_maps.append(m)

    res = run_bass_kernel_spmd(
        nc,
        in_maps,
        list(range(n_cores)),
        trace=trace,
        **(trace_kwargs or {}),
    )
    pts = np.concatenate([r["pts"] for r in res.results], axis=0)
    minn = np.concatenate(
        [r["minn"][:, 0] for r in res.results], axis=0
    )
    cub = np.concatenate([r["cub"] for r in res.results], axis=0)
    eye = np.broadcast_to(np.eye(3, dtype=np.float32), (bz, 3, 3)).copy()
    return (pts, minn, cub, eye), res


def kernel(**inputs):
    out, _ = _run(inputs, trace=False)
    return out


# revision 23
# speedup vs baseline: 4.3968x; 1.0470x over previous
"""Trainium2 Bass kernel for nn_DecoderFCWithCuboic.

Data-parallel over 8 NeuronCores: batch 4096 -> 512 rows/core, MLP weights
replicated. Per core:
  points branch: h1 = lrelu(x@W1+b1); h2 = lrelu(h1@W2+b2);
                 pts = sigmoid(h2@W3+b3)            (512, 6144)
  cuboid branch: cub = sigmoid(lrelu(lrelu(x@cW1+cb1)@cW2+cb2)@cW3+cb3)
  cuboid distance (per batch row b, point p):
     d'_w = c_w - s_w            (sign = inlier mask; exact under bf16)
     minn = mean_p( relu(min(min3(d'), min3(s))) + relu(max3(min(s, -BIG*d'))) )
  identities used:
     min3_w min(s, c-s) = min(min3(s), min3(c-s))
     s*[s>c] == relu(min(s, -BIG*(c-s)))   elementwise (BIG=1e30; any
       nonzero f32 d' has |d'|>=1e-8 here so BIG*|d'| >> 1, and d'==0
       gives 0 == the reference's inlier value)
     max3(s*[s>c]) is 0 when all coords are inliers, so the two terms
       just add -- no inlier-indicator select is needed.

The pts matmul runs in bf16 (W3 and h2 cast); everything else f32. The
cuboid-distance vector work runs in bf16 on DVE; d' is produced by the
Scalar engine (ACT) with the per-partition cub bias fused.
"""

import os
import sys

import numpy as np

for _p in ("/opt/trn_rl_repo", "/root/.axon_site/_ro/trn_rl_repo"):
    if os.path.isdir(_p) and _p not in sys.path:
        sys.path.insert(0, _p)

import ml_dtypes
import concourse.bass as bass  # noqa: F401
import concourse.mybir as mybir
import concourse.tile as tile
from concourse import bacc
from concourse.bass import ds, ts
from concourse.bass_utils import run_bass_kernel_spmd

BZ, D, H, NP = 4096, 128, 256, 2048  # batch, in-dim, hidden, points
NCORES = 8
BZC = BZ // NCORES  # 512 batch rows per core
NF = 3 * NP  # 6144 point features
F32 = mybir.dt.float32
BF = mybir.dt.bfloat16
EW = BF  # dtype for cuboid-distance intermediates
AF = mybir.ActivationFunctionType
OP = mybir.AluOpType
SLOPE = 0.01  # torch LeakyReLU default
BIG = 1.0e30


def _lrelu(nc, pool, z_psum, out_sbuf, bias_col):
    """out = lrelu(z + bias). ACT moves PSUM->SBUF with the per-partition bias
    folded in; DVE then applies max(x, SLOPE*x) in one fused op."""
    zsb = pool.tile(list(z_psum.shape), F32, tag="zsb")
    nc.scalar.activation(zsb, z_psum, AF.Identity, bias=bias_col, scale=1.0)
    nc.vector.scalar_tensor_tensor(
        out=out_sbuf, in0=zsb, scalar=SLOPE, in1=zsb, op0=OP.mult, op1=OP.max
    )


def _build(with_b3: bool, with_cb3: bool, n_rows: int = BZC):
    """Build the single-core SPMD program. n_rows must be a multiple of 128."""
    nc = bacc.Bacc()
    nbt = n_rows // 128  # batch sub-tiles

    xT_d = nc.dram_tensor("xT", [D, n_rows], F32, kind="ExternalInput")
    W1_d = nc.dram_tensor("W1", [D, H], F32, kind="ExternalInput")
    b1_d = nc.dram_tensor("b1", [H], F32, kind="ExternalInput")
    W2_d = nc.dram_tensor("W2", [H, H], F32, kind="ExternalInput")
    b2_d = nc.dram_tensor("b2", [H], F32, kind="ExternalInput")
    W3_d = nc.dram_tensor("W3", [H, NF], BF, kind="ExternalInput")
    b3_d = nc.dram_tensor("b3", [1, NF], BF, kind="ExternalInput")
    cW1_d = nc.dram_tensor("cW1", [D, H], F32, kind="ExternalInput")
    cb1_d = nc.dram_tensor("cb1", [H], F32, kind="ExternalInput")
    cW2_d = nc.dram_tensor("cW2", [H, H], F32, kind="ExternalInput")
    cb2_d = nc.dram_tensor("cb2", [H], F32, kind="ExternalInput")
    cW3_d = nc.dram_tensor("cW3", [H, 3], F32, kind="ExternalInput")
    cb3_d = nc.dram_tensor("cb3", [1, 3], F32, kind="ExternalInput")

    pts_d = nc.dram_tensor("pts", [n_rows, 3, NP], F32, kind="ExternalOutput")
    minn_d = nc.dram_tensor("minn", [n_rows, 1], F32, kind="ExternalOutput")
    cub_d = nc.dram_tensor("cub", [n_rows, 3], F32, kind="ExternalOutput")

    from contextlib import ExitStack

    with tile.TileContext(nc) as tc, ExitStack() as ctx:
        consts = ctx.enter_context(tc.tile_pool(name="consts", bufs=1))

        # ---- resident weights / biases -------------------------------------
        xT = consts.tile([D, n_rows], F32)
        nc.sync.dma_start(xT, xT_d[:, :])
        W1 = consts.tile([128, H], F32)
        nc.sync.dma_start(W1, W1_d[:, :])
        W2 = consts.tile([128, 2, H], F32)
        nc.sync.dma_start(W2, W2_d[:, :].rearrange("(k p) m -> p k m", p=128))
        cW1 = consts.tile([128, H], F32)
        nc.sync.dma_start(cW1, cW1_d[:, :])
        cW2 = consts.tile([128, 2, H], F32)
        nc.sync.dma_start(cW2, cW2_d[:, :].rearrange("(k p) m -> p k m", p=128))
        cW3 = consts.tile([128, 2, 3], F32)
        nc.sync.dma_start(cW3, cW3_d[:, :].rearrange("(k p) n -> p k n", p=128))
        b1 = consts.tile([128, 2], F32)
        nc.sync.dma_start(b1, b1_d[:].rearrange("(m p) -> p m", p=128))
        b2 = consts.tile([128, 2], F32)
        nc.sync.dma_start(b2, b2_d[:].rearrange("(m p) -> p m", p=128))
        cb1 = consts.tile([128, 2], F32)
        nc.sync.dma_start(cb1, cb1_d[:].rearrange("(m p) -> p m", p=128))
        cb2 = consts.tile([128, 2], F32)
        nc.sync.dma_start(cb2, cb2_d[:].rearrange("(m p) -> p m", p=128))
        cb3 = consts.tile([1, 3], F32)
        nc.sync.dma_start(cb3, cb3_d[:, :])
        # W3 is the big (3.1 MB) transfer: issue it after the small weights so
        # the layer phase isn't queued behind it.
        W3 = consts.tile([128, 2, NF], BF)
        nc.sync.dma_start(W3, W3_d[:, :].rearrange("(k p) n -> p k n", p=128))
        ones = consts.tile([1, 128], F32)
        nc.vector.memset(ones, 1.0)
        b3 = None
        onesb = None
        if with_b3:
            b3 = consts.tile([1, NF], BF)
            nc.sync.dma_start(b3, b3_d[:, :])
            onesb = consts.tile([1, 128], BF)
            nc.vector.memset(onesb, 1.0)

        # ---- layer phase: h1/h2/c1/c2, feature-major [feat, batch] ---------
        hpool = ctx.enter_context(tc.tile_pool(name="hpool", bufs=1))
        h2 = hpool.tile([128, 2, n_rows], BF)
        c2 = hpool.tile([128, 2, n_rows], F32)
        with (
            tc.tile_pool(name="lay_psum", bufs=4, space="PSUM") as lpsum,
            tc.tile_pool(name="zpool", bufs=2) as zpool,
            tc.tile_pool(name="h1pool", bufs=1) as h1pool,
        ):
            h1 = h1pool.tile([128, 2, n_rows], F32)
            c1 = h1pool.tile([128, 2, n_rows], F32)
            for W_in, b_in, out_t in ((W1, b1, h1), (cW1, cb1, c1)):
                for m in range(2):
                    zp = lpsum.tile([128, n_rows], F32, tag="zp")
                    nc.tensor.matmul(
                        zp, W_in[:, ts(m, 128)], xT[:, :], start=True, stop=True
                    )
                    _lrelu(nc, zpool, zp, out_t[:, m, :], b_in[:, m : m + 1])
            for W_in, b_in, in_t, out_t in ((W2, b2, h1, h2), (cW2, cb2, c1, c2)):
                for m in range(2):
                    zp = lpsum.tile([128, n_rows], F32, tag="zp")
                    for k in range(2):
                        nc.tensor.matmul(
                            zp,
                            W_in[:, k, ts(m, 128)],
                            in_t[:, k, :],
                            start=(k == 0),
                            stop=(k == 1),
                        )
                    _lrelu(nc, zpool, zp, out_t[:, m, :], b_in[:, m : m + 1])

        # ---- per-sub-tile phase --------------------------------------------
        spool = ctx.enter_context(tc.tile_pool(name="spool", bufs=2))
        dspool = ctx.enter_context(tc.tile_pool(name="dspool", bufs=2))
        epool = ctx.enter_context(tc.tile_pool(name="epool", bufs=1))
        opool = ctx.enter_context(tc.tile_pool(name="opool", bufs=2))
        tpool = ctx.enter_context(tc.tile_pool(name="tpool", bufs=3))
        cpool = ctx.enter_context(tc.tile_pool(name="cpool", bufs=2))
        smpool = ctx.enter_context(tc.tile_pool(name="smpool", bufs=4))
        pts_psum = ctx.enter_context(
            tc.tile_pool(name="pts_psum", bufs=2, space="PSUM")
        )

        for bt in range(nbt):
            bsl = ts(bt, 128)
            # cuboid head: cub[128, 3] (batch-partition layout); borrows a
            # pts_psum slot.
            cp = pts_psum.tile([128, 1024], F32, tag="pp")
            if with_cb3:
                nc.tensor.matmul(cp[:, 0:3], ones, cb3, start=True, stop=False)
            for k in range(2):
                nc.tensor.matmul(
                    cp[:, 0:3],
                    c2[:, k, bsl],
                    cW3[:, k, :],
                    start=(k == 0 and not with_cb3),
                    stop=(k == 1),
                )
            cub_s = cpool.tile([128, 3], F32, tag="cub")
            nc.scalar.activation(cub_s, cp[:, 0:3], AF.Sigmoid)
            nc.sync.dma_start(cub_d[bsl, :], cub_s)

            # pts matmuls (bf16) + sigmoid: s [128, 3, 2048] f32
            s = spool.tile([128, 3, NP], F32, tag="s")
            for w in range(3):
                for half in range(2):
                    pp = pts_psum.tile([128, 1024], F32, tag="pp")
                    col0 = w * NP + half * 1024
                    if with_b3:
                        for nn in range(2):
                            nc.tensor.matmul(
                                pp[:, ts(nn, 512)],
                                onesb,
                                b3[:, ds(col0 + nn * 512, 512)],
                                start=True,
                                stop=False,
                            )
                    for k in range(2):
                        for nn in range(2):
                            nc.tensor.matmul(
                                pp[:, ts(nn, 512)],
                                h2[:, k, bsl],
                                W3[:, k, ds(col0 + nn * 512, 512)],
                                start=(k == 0 and not with_b3),
                                stop=(k == 1),
                            )
                    nc.scalar.activation(
                        s[:, w, ds(half * 1024, 1024)], pp, AF.Sigmoid
                    )
            nc.sync.dma_start(pts_d[bsl, :, :], s)

            # dst holds [d'_0, d'_1, d'_2, s_0, s_1, s_2] in bf16 so the min
            # trees can run two lanes per op via strided views.
            dst = dspool.tile([128, 6, NP], EW, tag="dst")
            nc.vector.tensor_copy(out=dst[:, 3:6, :], in_=s)
            for w in range(3):
                nc.scalar.activation(
                    dst[:, w, :],
                    s[:, w, :],
                    AF.Identity,
                    bias=cub_s[:, w : w + 1],
                    scale=-1.0,
                )

            # O-side: o = min(s, -BIG*d'), Oterm = max3(o)
            eh = epool.tile([128, 3, NP], EW, tag="e")
            nc.vector.tensor_scalar(
                out=eh, in0=dst[:, 0:3, :], scalar1=-BIG, scalar2=None,
                op0=OP.mult,
            )
            o_t = opool.tile([128, 3, NP], EW, tag="o")
            nc.vector.tensor_tensor(o_t, dst[:, 3:6, :], eh, OP.min)
            t1 = tpool.tile([128, NP], EW, tag="t")
            nc.vector.tensor_tensor(t1, o_t[:, 0, :], o_t[:, 1, :], OP.max)
            t2 = tpool.tile([128, NP], EW, tag="t")
            nc.vector.tensor_tensor(t2, t1, o_t[:, 2, :], OP.max)

            # R-side: Rterm = min(min3(d'), min3(s)), two lanes per op via the
            # (g w) view of dst.  Rterm > 0 only when all coords are inliers,
            # and then Oterm <= 0 -- mutually exclusive, so one fused relu+sum:
            #   result = relu(max(Rterm, Oterm))
            dsv = dst.rearrange("p (g w) n -> p w g n", g=2)
            uv1 = tpool.tile([128, 2, NP], EW, tag="uv")
            nc.vector.tensor_tensor(uv1, dsv[:, 0, :, :], dsv[:, 1, :, :], OP.min)
            uv2 = tpool.tile([128, 2, NP], EW, tag="uv")
            nc.vector.tensor_tensor(uv2, uv1, dsv[:, 2, :, :], OP.min)
            q1 = tpool.tile([128, NP], EW, tag="t")
            nc.vector.tensor_tensor(q1, uv2[:, 0, :], uv2[:, 1, :], OP.min)
            fin = tpool.tile([128, NP], EW, tag="t")
            nc.vector.tensor_tensor(fin, q1, t2, OP.max)
            msum = smpool.tile([128, 1], F32, tag="msum")
            f2 = tpool.tile([128, NP], EW, tag="t")
            nc.vector.tensor_scalar(
                out=f2, in0=fin, scalar1=0.0, scalar2=0.0,
                op0=OP.max, op1=OP.add, accum_out=msum,
            )

            minn_s = smpool.tile([128, 1], F32, tag="minn")
            nc.vector.tensor_scalar_mul(minn_s, msum, 1.0 / NP)
            nc.sync.dma_start(minn_d[bsl, :], minn_s)

    return nc


def _run(inputs, trace=False, n_rows=BZC, n_cores=NCORES, trace_kwargs=None):
    """Shard inputs, build + run the SPMD program, gather outputs.

    Returns (outputs_tuple, BassKernelResults).
    """
    f = lambda a: np.asarray(a, dtype=np.float32)
    x = f(inputs["x"])
    bz = x.shape[0]
    assert bz == n_rows * n_cores
    names = [
        "W1", "b1", "W2", "b2", "W3", "b3", "cW1", "cb1", "cW2", "cb2", "cW3",
        "cb3",
    ]
    ws = {k: f(inputs[k]) for k in names}
    ws["W3"] = ws["W3"].astype(ml_dtypes.bfloat16)
    ws["b3"] = ws["b3"].reshape(1, NF).astype(ml_dtypes.bfloat16)
    ws["cb3"] = ws["cb3"].reshape(1, 3)
    with_b3 = bool(np.any(inputs["b3"]))
    with_cb3 = bool(np.any(ws["cb3"]))

    nc = _build(with_b3, with_cb3, n_rows=n_rows)
    nc.finalize()  # Bacc: runs wait-splitting + regalloc passes

    in_maps = []
    for i in range(n_cores):
        xs = x[i * n_rows : (i + 1) * n_rows]
        m = {"xT": np.ascontiguousarray(xs.T)}
        m.update(ws)
        in_maps.append(m)

    res = run_bass_kernel_spmd(
        nc,
        in_maps,
        list(range(n_cores)),
        trace=trace,
        **(trace_kwargs or {}),
    )
    pts = np.concatenate([r["pts"] for r in res.results], axis=0)
    minn = np.concatenate(
        [r["minn"][:, 0] for r in res.results], axis=0
    )
    cub = np.concatenate([r["cub"] for r in res.results], axis=0)
    eye = np.broadcast_to(np.eye(3, dtype=np.float32), (bz, 3, 3)).copy()
    return (pts, minn, cub, eye), res


def kernel(**inputs):
    out, _ = _run(inputs, trace=False)
    return out


# revision 24
# speedup vs baseline: 4.5687x; 1.0391x over previous
"""Trainium2 Bass kernel for nn_DecoderFCWithCuboic.

Data-parallel over 8 NeuronCores: batch 4096 -> 512 rows/core, MLP weights
replicated. Per core:
  points branch: h1 = lrelu(x@W1+b1); h2 = lrelu(h1@W2+b2);
                 pts = sigmoid(h2@W3+b3)            (512, 6144)
  cuboid branch: cub = sigmoid(lrelu(lrelu(x@cW1+cb1)@cW2+cb2)@cW3+cb3)
  cuboid distance (per batch row b, point p):
     d'_w = c_w - s_w            (sign = inlier mask; exact under bf16)
     minn = mean_p( relu(min(min3(d'), min3(s))) + relu(max3(min(s, -BIG*d'))) )
  identities used:
     min3_w min(s, c-s) = min(min3(s), min3(c-s))
     s*[s>c] == relu(min(s, -BIG*(c-s)))   elementwise (BIG=1e30; any
       nonzero f32 d' has |d'|>=1e-8 here so BIG*|d'| >> 1, and d'==0
       gives 0 == the reference's inlier value)
     max3(s*[s>c]) is 0 when all coords are inliers, so the two terms
       just add -- no inlier-indicator select is needed.

The pts matmul runs in bf16 (W3 and h2 cast); everything else f32. The
cuboid-distance vector work runs in bf16 on DVE; d' is produced by the
Scalar engine (ACT) with the per-partition cub bias fused.
"""

import os
import sys

import numpy as np

for _p in ("/opt/trn_rl_repo", "/root/.axon_site/_ro/trn_rl_repo"):
    if os.path.isdir(_p) and _p not in sys.path:
        sys.path.insert(0, _p)

import ml_dtypes
import concourse.bass as bass  # noqa: F401
import concourse.mybir as mybir
import concourse.tile as tile
from concourse import bacc
from concourse.bass import ds, ts
from concourse.bass_utils import run_bass_kernel_spmd

BZ, D, H, NP = 4096, 128, 256, 2048  # batch, in-dim, hidden, points
NCORES = 8
BZC = BZ // NCORES  # 512 batch rows per core
NF = 3 * NP  # 6144 point features
F32 = mybir.dt.float32
BF = mybir.dt.bfloat16
EW = BF  # dtype for cuboid-distance intermediates
AF = mybir.ActivationFunctionType
OP = mybir.AluOpType
SLOPE = 0.01  # torch LeakyReLU default
BIG = 1.0e30


def _lrelu(nc, pool, z_psum, out_sbuf, bias_col):
    """out = lrelu(z + bias). ACT moves PSUM->SBUF with the per-partition bias
    folded in; DVE then applies max(x, SLOPE*x) in one fused op."""
    zsb = pool.tile(list(z_psum.shape), F32, tag="zsb")
    nc.scalar.activation(zsb, z_psum, AF.Identity, bias=bias_col, scale=1.0)
    nc.vector.scalar_tensor_tensor(
        out=out_sbuf, in0=zsb, scalar=SLOPE, in1=zsb, op0=OP.mult, op1=OP.max
    )


def _build(with_b3: bool, with_cb3: bool, n_rows: int = BZC):
    """Build the single-core SPMD program. n_rows must be a multiple of 128."""
    nc = bacc.Bacc()
    nbt = n_rows // 128  # batch sub-tiles

    xT_d = nc.dram_tensor("xT", [D, n_rows], F32, kind="ExternalInput")
    W1_d = nc.dram_tensor("W1", [D, H], F32, kind="ExternalInput")
    b1_d = nc.dram_tensor("b1", [H], F32, kind="ExternalInput")
    W2_d = nc.dram_tensor("W2", [H, H], F32, kind="ExternalInput")
    b2_d = nc.dram_tensor("b2", [H], F32, kind="ExternalInput")
    W3_d = nc.dram_tensor("W3", [H, NF], BF, kind="ExternalInput")
    b3_d = nc.dram_tensor("b3", [1, NF], BF, kind="ExternalInput")
    cW1_d = nc.dram_tensor("cW1", [D, H], F32, kind="ExternalInput")
    cb1_d = nc.dram_tensor("cb1", [H], F32, kind="ExternalInput")
    cW2_d = nc.dram_tensor("cW2", [H, H], F32, kind="ExternalInput")
    cb2_d = nc.dram_tensor("cb2", [H], F32, kind="ExternalInput")
    cW3_d = nc.dram_tensor("cW3", [H, 3], F32, kind="ExternalInput")
    cb3_d = nc.dram_tensor("cb3", [1, 3], F32, kind="ExternalInput")

    pts_d = nc.dram_tensor("pts", [n_rows, 3, NP], F32, kind="ExternalOutput")
    minn_d = nc.dram_tensor("minn", [n_rows, 1], F32, kind="ExternalOutput")
    cub_d = nc.dram_tensor("cub", [n_rows, 3], F32, kind="ExternalOutput")

    from contextlib import ExitStack

    with tile.TileContext(nc) as tc, ExitStack() as ctx:
        consts = ctx.enter_context(tc.tile_pool(name="consts", bufs=1))

        # ---- resident weights / biases -------------------------------------
        xT = consts.tile([D, n_rows], F32)
        nc.sync.dma_start(xT, xT_d[:, :])
        W1 = consts.tile([128, H], F32)
        nc.sync.dma_start(W1, W1_d[:, :])
        W2 = consts.tile([128, 2, H], F32)
        nc.sync.dma_start(W2, W2_d[:, :].rearrange("(k p) m -> p k m", p=128))
        cW1 = consts.tile([128, H], F32)
        nc.sync.dma_start(cW1, cW1_d[:, :])
        cW2 = consts.tile([128, 2, H], F32)
        nc.sync.dma_start(cW2, cW2_d[:, :].rearrange("(k p) m -> p k m", p=128))
        cW3 = consts.tile([128, 2, 3], F32)
        nc.sync.dma_start(cW3, cW3_d[:, :].rearrange("(k p) n -> p k n", p=128))
        b1 = consts.tile([128, 2], F32)
        nc.sync.dma_start(b1, b1_d[:].rearrange("(m p) -> p m", p=128))
        b2 = consts.tile([128, 2], F32)
        nc.sync.dma_start(b2, b2_d[:].rearrange("(m p) -> p m", p=128))
        cb1 = consts.tile([128, 2], F32)
        nc.sync.dma_start(cb1, cb1_d[:].rearrange("(m p) -> p m", p=128))
        cb2 = consts.tile([128, 2], F32)
        nc.sync.dma_start(cb2, cb2_d[:].rearrange("(m p) -> p m", p=128))
        cb3 = consts.tile([1, 3], F32)
        nc.sync.dma_start(cb3, cb3_d[:, :])
        # W3 is the big (3.1 MB) transfer: issue it after the small weights so
        # the layer phase isn't queued behind it.
        W3 = consts.tile([128, 2, NF], BF)
        nc.sync.dma_start(W3, W3_d[:, :].rearrange("(k p) n -> p k n", p=128))
        ones = consts.tile([1, 128], F32)
        nc.vector.memset(ones, 1.0)
        b3 = None
        onesb = None
        if with_b3:
            b3 = consts.tile([1, NF], BF)
            nc.sync.dma_start(b3, b3_d[:, :])
            onesb = consts.tile([1, 128], BF)
            nc.vector.memset(onesb, 1.0)

        # ---- layer phase: h1/h2/c1/c2, feature-major [feat, batch] ---------
        hpool = ctx.enter_context(tc.tile_pool(name="hpool", bufs=1))
        h2 = hpool.tile([128, 2, n_rows], BF)
        c2 = hpool.tile([128, 2, n_rows], F32)
        with (
            tc.tile_pool(name="lay_psum", bufs=4, space="PSUM") as lpsum,
            tc.tile_pool(name="zpool", bufs=2) as zpool,
            tc.tile_pool(name="h1pool", bufs=1) as h1pool,
        ):
            h1 = h1pool.tile([128, 2, n_rows], F32)
            c1 = h1pool.tile([128, 2, n_rows], F32)
            for W_in, b_in, out_t in ((W1, b1, h1), (cW1, cb1, c1)):
                for m in range(2):
                    zp = lpsum.tile([128, n_rows], F32, tag="zp")
                    nc.tensor.matmul(
                        zp, W_in[:, ts(m, 128)], xT[:, :], start=True, stop=True
                    )
                    _lrelu(nc, zpool, zp, out_t[:, m, :], b_in[:, m : m + 1])
            for W_in, b_in, in_t, out_t in ((W2, b2, h1, h2), (cW2, cb2, c1, c2)):
                for m in range(2):
                    zp = lpsum.tile([128, n_rows], F32, tag="zp")
                    for k in range(2):
                        nc.tensor.matmul(
                            zp,
                            W_in[:, k, ts(m, 128)],
                            in_t[:, k, :],
                            start=(k == 0),
                            stop=(k == 1),
                        )
                    _lrelu(nc, zpool, zp, out_t[:, m, :], b_in[:, m : m + 1])

        # ---- per-sub-tile phase --------------------------------------------
        spool = ctx.enter_context(tc.tile_pool(name="spool", bufs=2))
        dspool = ctx.enter_context(tc.tile_pool(name="dspool", bufs=2))
        epool = ctx.enter_context(tc.tile_pool(name="epool", bufs=1))
        opool = ctx.enter_context(tc.tile_pool(name="opool", bufs=2))
        tpool = ctx.enter_context(tc.tile_pool(name="tpool", bufs=3))
        cpool = ctx.enter_context(tc.tile_pool(name="cpool", bufs=2))
        smpool = ctx.enter_context(tc.tile_pool(name="smpool", bufs=4))
        pts_psum = ctx.enter_context(
            tc.tile_pool(name="pts_psum", bufs=2, space="PSUM")
        )

        for bt in range(nbt):
            bsl = ts(bt, 128)
            # cuboid head: cub[128, 3] (batch-partition layout); borrows a
            # pts_psum slot.
            cp = pts_psum.tile([128, 1024], F32, tag="pp")
            if with_cb3:
                nc.tensor.matmul(cp[:, 0:3], ones, cb3, start=True, stop=False)
            for k in range(2):
                nc.tensor.matmul(
                    cp[:, 0:3],
                    c2[:, k, bsl],
                    cW3[:, k, :],
                    start=(k == 0 and not with_cb3),
                    stop=(k == 1),
                )
            cub_s = cpool.tile([128, 3], F32, tag="cub")
            nc.scalar.activation(cub_s, cp[:, 0:3], AF.Sigmoid)
            nc.sync.dma_start(cub_d[bsl, :], cub_s)

            # pts matmuls (bf16) + sigmoid: s [128, 3, 2048] f32
            s = spool.tile([128, 3, NP], F32, tag="s")
            for w in range(3):
                for half in range(2):
                    pp = pts_psum.tile([128, 1024], F32, tag="pp")
                    col0 = w * NP + half * 1024
                    if with_b3:
                        for nn in range(2):
                            nc.tensor.matmul(
                                pp[:, ts(nn, 512)],
                                onesb,
                                b3[:, ds(col0 + nn * 512, 512)],
                                start=True,
                                stop=False,
                            )
                    for k in range(2):
                        for nn in range(2):
                            nc.tensor.matmul(
                                pp[:, ts(nn, 512)],
                                h2[:, k, bsl],
                                W3[:, k, ds(col0 + nn * 512, 512)],
                                start=(k == 0 and not with_b3),
                                stop=(k == 1),
                            )
                    nc.scalar.activation(
                        s[:, w, ds(half * 1024, 1024)], pp, AF.Sigmoid
                    )
            nc.sync.dma_start(pts_d[bsl, :, :], s)

            # dst holds [d'_0, d'_1, d'_2, s_0, s_1, s_2] in bf16 so the min
            # trees can run two lanes per op via strided views.
            dst = dspool.tile([128, 6, NP], EW, tag="dst")
            nc.vector.tensor_copy(out=dst[:, 3:6, :], in_=s)
            for w in range(3):
                nc.scalar.activation(
                    dst[:, w, :],
                    s[:, w, :],
                    AF.Identity,
                    bias=cub_s[:, w : w + 1],
                    scale=-1.0,
                )

            # O-side: o = min(s, -BIG*d'), Oterm = max3(o)
            eh = epool.tile([128, 3, NP], EW, tag="e")
            nc.vector.tensor_scalar(
                out=eh, in0=dst[:, 0:3, :], scalar1=-BIG, scalar2=None,
                op0=OP.mult,
            )
            o_t = opool.tile([128, 3, NP], EW, tag="o")
            nc.vector.tensor_tensor(o_t, dst[:, 3:6, :], eh, OP.min)
            t1 = tpool.tile([128, NP], EW, tag="t")
            nc.vector.tensor_tensor(t1, o_t[:, 0, :], o_t[:, 1, :], OP.max)
            t2 = tpool.tile([128, NP], EW, tag="t")
            nc.vector.tensor_tensor(t2, t1, o_t[:, 2, :], OP.max)

            # R-side: Rterm = min(min3(d'), min3(s)), two lanes per op via the
            # (g w) view of dst.  Rterm > 0 only when all coords are inliers,
            # and then Oterm <= 0 -- mutually exclusive, so one fused relu+sum:
            #   result = relu(max(Rterm, Oterm))
            dsv = dst.rearrange("p (g w) n -> p w g n", g=2)
            uv1 = tpool.tile([128, 2, NP], EW, tag="uv")
            nc.vector.tensor_tensor(uv1, dsv[:, 0, :, :], dsv[:, 1, :, :], OP.min)
            uv2 = tpool.tile([128, 2, NP], EW, tag="uv")
            nc.vector.tensor_tensor(uv2, uv1, dsv[:, 2, :, :], OP.min)
            q1 = tpool.tile([128, NP], EW, tag="t")
            nc.vector.tensor_tensor(q1, uv2[:, 0, :], uv2[:, 1, :], OP.min)
            fin = tpool.tile([128, NP], EW, tag="t")
            nc.vector.tensor_tensor(fin, q1, t2, OP.max)
            msum = smpool.tile([128, 1], F32, tag="msum")
            f2 = tpool.tile([128, NP], EW, tag="t")
            nc.scalar.activation(
                f2, fin, AF.Relu, bias=0.0, scale=1.0, accum_out=msum
            )

            minn_s = smpool.tile([128, 1], F32, tag="minn")
            nc.vector.tensor_scalar_mul(minn_s, msum, 1.0 / NP)
            nc.sync.dma_start(minn_d[bsl, :], minn_s)

    return nc


def _run(inputs, trace=False, n_rows=BZC, n_cores=NCORES, trace_kwargs=None):
    """Shard inputs, build + run the SPMD program, gather outputs.

    Returns (outputs_tuple, BassKernelResults).
    """
    f = lambda a: np.asarray(a, dtype=np.float32)
    x = f(inputs["x"])
    bz = x.shape[0]
    assert bz == n_rows * n_cores
    names = [
        "W1", "b1", "W2", "b2", "W3", "b3", "cW1", "cb1", "cW2", "cb2", "cW3",
        "cb3",
    ]
    ws = {k: f(inputs[k]) for k in names}
    ws["W3"] = ws["W3"].astype(ml_dtypes.bfloat16)
    ws["b3"] = ws["b3"].reshape(1, NF).astype(ml_dtypes.bfloat16)
    ws["cb3"] = ws["cb3"].reshape(1, 3)
    with_b3 = bool(np.any(inputs["b3"]))
    with_cb3 = bool(np.any(ws["cb3"]))

    nc = _build(with_b3, with_cb3, n_rows=n_rows)
    nc.finalize()  # Bacc: runs wait-splitting + regalloc passes

    in_maps = []
    for i in range(n_cores):
        xs = x[i * n_rows : (i + 1) * n_rows]
        m = {"xT": np.ascontiguousarray(xs.T)}
        m.update(ws)
        in_maps.append(m)

    res = run_bass_kernel_spmd(
        nc,
        in_maps,
        list(range(n_cores)),
        trace=trace,
        **(trace_kwargs or {}),
    )
    pts = np.concatenate([r["pts"] for r in res.results], axis=0)
    minn = np.concatenate(
        [r["minn"][:, 0] for r in res.results], axis=0
    )
    cub = np.concatenate([r["cub"] for r in res.results], axis=0)
    eye = np.broadcast_to(np.eye(3, dtype=np.float32), (bz, 3, 3)).copy()
    return (pts, minn, cub, eye), res


def kernel(**inputs):
    out, _ = _run(inputs, trace=False)
    return out
